# revision 1
# baseline (speedup 1.0000x reference)
"""Trainium2 Bass kernel for nn_DecoderBlock (BitNet-style decoder block with
self-attention, cross-attention and BitFeedForward), data-parallel over
(batch x sequence) tokens across 8 NeuronCores.

Sharding: 4096 tokens (B=2 x N=2048) split into 8 shards of 512 tokens.
Cores 0-3 hold batch 0, cores 4-7 batch 1. Self-attention K/V are computed
on local tokens and AllGather-ed within each 4-core batch group. Everything
else (cross-attention over the 256 condition tokens, FFN, projections) is
fully local; weights are replicated. Weights are staged host-side in
transposed layout ([in, out]) so the contraction dim lands on partitions
without any device-side transposes; all quantization runs on device.

Numerics: BitLinear activation/weight fake-quantization is computed in the
integer domain; integer-valued operands are exact in bf16, so the bf16
matmul path is exact for the quantized matmuls (fp32 PSUM accumulation).
Attention score / PV matmuls run in bf16 (validated at the fp32
reimplementation noise floor).
"""

import numpy as np
from contextlib import ExitStack

import concourse.bacc as bacc
import concourse.bass as bass
import concourse.mybir as mybir
import concourse.tile as tile
from concourse.bass_utils import run_bass_kernel_spmd
from concourse.masks import make_identity

F32 = mybir.dt.float32
BF16 = mybir.dt.bfloat16
I8 = mybir.dt.int8
I16 = mybir.dt.int16
AX = mybir.AxisListType
OP = mybir.AluOpType
ACT = mybir.ActivationFunctionType

# model dims
B, N, S, D = 2, 2048, 256, 768
HQ, HK, HEAD = 12, 6, 64
DKV = HEAD * HK          # 384
H4 = 4 * D               # 3072
NCORES = 8
GROUPS = [[0, 1, 2, 3], [4, 5, 6, 7]]
GSZ = 4                  # cores per batch group
T = (B * N) // NCORES    # 512 tokens per core
NT = T // 128            # 4 token tiles per core
ST = S // 128            # 2 condition token tiles
KT = D // 128            # 6 feature tiles of D
KTH = H4 // 128          # 24 feature tiles of 4D

# (out_features, in_features) of each BitLinear weight; the device receives
# the host-transposed [in, out] layout as parameter f"{name}_t".
WSPECS = {
    'sa_wq': (D, D), 'sa_wk': (DKV, D), 'sa_wv': (DKV, D), 'sa_wo': (D, D),
    'ca_wq': (D, D), 'ca_wk': (DKV, D), 'ca_wv': (DKV, D), 'ca_wo': (D, D),
    'w_cond': (D, D), 'w1': (H4, D), 'w2': (D, H4),
}

_PROGRAM_CACHE = {}

# CoreSim's float->int convert truncates; real HW rounds (round-to-nearest).
# Build the rounding idiom accordingly.
SIM_COMPAT = False


class Ctx:
    """Shared build-state: nc plus long-lived pools/constants."""
    pass


def _rsqrt(g, out, in_, eps_tile, tmp):
    """out = (in_ + eps)^-1/2 via exp(-0.5*ln(.)) -- stays in the
    natural_log_exp ACT table set shared with softmax's exp."""
    g.nc.scalar.activation(tmp, in_, ACT.Ln, bias=eps_tile)
    g.nc.scalar.activation(out, tmp, ACT.Exp, bias=0.0, scale=-0.5)


def _prep_weight(g, name, pool, wt_in, wwork, two_pass=False):
    """Quantize one host-transposed weight [I, O] to ternary bf16 tiles.

    Returns (list of I//128 tiles [128, O] bf16, m_bcast [128,1]) where
    m = clip(mean|w|, 1e-5) is the dequant multiplier. two_pass re-DMAs the
    fp32 tiles during quantization instead of holding them all in SBUF.
    """
    nc, stat = g.nc, g.stat
    O, I = WSPECS[name]
    rows = I // 128
    with g.tc.tile_pool(name=f"wf_{name}", bufs=(2 if two_pass else 1)) as wf:
        colsum = stat.tile([128, rows], F32, tag="colsum", name="colsum")
        wtiles = []
        for r in range(rows):
            wt = wf.tile([128, O], F32,
                         tag=("w" if two_pass else f"w{r}"), name=f"w{r}")
            nc.sync.dma_start(wt, wt_in[r * 128:(r + 1) * 128, :])
            nc.vector.tensor_reduce(colsum[:, r:r + 1], wt, axis=AX.X,
                                    op=OP.add, apply_absolute_value=True)
            if not two_pass:
                wtiles.append(wt)
        asum = stat.tile([128, 1], F32, tag="s1", name="s1")
        nc.vector.tensor_reduce(asum, colsum, axis=AX.X, op=OP.add)
        ps1 = g.psum.tile([1, 1], F32, tag="ps", name="ps1")
        nc.tensor.matmul(ps1, asum, g.ones_col, start=True, stop=True)
        m = stat.tile([1, 1], F32, tag="m0", name="m0")
        nc.scalar.activation(m, ps1, ACT.Copy, bias=0.0,
                             scale=1.0 / float(O * I))
        nc.vector.tensor_scalar_max(m, m, 1e-5)
        mb = g.const.tile([128, 1], F32, tag=f"mb_{name}", name=f"mb_{name}")
        nc.gpsimd.partition_broadcast(mb, m[0:1, :])
        invm = stat.tile([1, 1], F32, tag="m1", name="m1")
        nc.vector.reciprocal(invm, m)
        imb = stat.tile([128, 1], F32, tag="m2", name="m2")
        nc.gpsimd.partition_broadcast(imb, invm[0:1, :])
        # ternary quant: clip(round(w/m),-1,1); HW's fp32->int8 convert
        # rounds to nearest (sim: truncates, hence the 2x trick there).
        out_tiles = []
        for r in range(rows):
            if two_pass:
                wt = wf.tile([128, O], F32, tag="w", name=f"wb{r}")
                nc.sync.dma_start(wt, wt_in[r * 128:(r + 1) * 128, :])
            else:
                wt = wtiles[r]
            i8 = wwork.tile([128, O], I8, tag=f"i8_{O}", name="i8", bufs=2)
            if SIM_COMPAT:
                nc.vector.tensor_scalar(i8, wt, imb, 2.0,
                                        OP.mult, OP.mult)
            else:
                nc.vector.tensor_scalar_mul(i8, wt, imb)
            wq = pool.tile([128, O], BF16, tag=f"wq_{name}_{r}",
                           name=f"wq_{name}_{r}")
            nc.gpsimd.tensor_scalar(wq, i8, -1.0, 1.0, OP.max, OP.min)
            out_tiles.append(wq)
    return out_tiles, mb


def _act_quant(g, x_tiles, F, xq_pool, work, make_row, uid):
    """BitLinear input processing: fused RMSNorm + per-token int8 absmax
    quant, emitting feature-major integer-valued bf16 tiles via DMA-transpose.

    Returns (xqT: F//128 tiles [128, Ttot] bf16, alphas: [128,1] per token
    tile, alpha_bcast: [128, Ttot] row-broadcast or None).
    alpha = clip(absmax(x)*rsqrt(mean(x^2)+1e-6), 1e-5)/127; multiplying the
    integer matmul result by alpha*mean|w| dequantizes bitlinear().
    """
    nc, stat = g.nc, g.stat
    nj = len(x_tiles)
    Ttot = nj * 128
    FK = F // 128
    sub = 256 if F % 512 else 512
    ns = F // sub
    xqT = [xq_pool.tile([128, Ttot], BF16, tag=f"xqT_{uid}_{k}",
                        name=f"xqT_{uid}_{k}") for k in range(FK)]
    alphas = []
    for j, X in enumerate(x_tiles):
        stats = stat.tile([128, ns, 6], F32, tag="bnst", name="bnst")
        Xg = X.rearrange("p (n s) -> p n s", s=sub)
        for gi in range(ns):
            nc.vector.bn_stats(stats[:, gi, :], Xg[:, gi, :])
        mv = stat.tile([128, 2], F32, tag="mv", name="mv")
        nc.vector.bn_aggr(mv, stats)
        ms = stat.tile([128, 1], F32, tag="s1", name="s1")
        nc.vector.tensor_mul(ms, mv[:, 0:1], mv[:, 0:1])
        nc.vector.tensor_add(ms, ms, mv[:, 1:2])
        r = stat.tile([128, 1], F32, tag="s2", name="s2")
        t0 = stat.tile([128, 1], F32, tag="s3", name="s3")
        _rsqrt(g, r, ms, g.eps6, t0)
        amax = stat.tile([128, 1], F32, tag="s4", name="s4")
        nc.vector.tensor_reduce(amax, X, axis=AX.X, op=OP.max,
                                apply_absolute_value=True)
        amn = stat.tile([128, 1], F32, tag="s5", name="s5")
        nc.vector.tensor_mul(amn, amax, r)
        nc.vector.tensor_scalar_max(amn, amn, 1e-5)
        al = stat.tile([128, 1], F32, tag=f"al_{uid}_{j}",
                       name=f"al_{uid}_{j}")
        nc.vector.tensor_scalar_mul(al, amn, 1.0 / 127.0)
        alphas.append(al)
        ra = stat.tile([128, 1], F32, tag="s6", name="s6")
        nc.vector.reciprocal(ra, amn)
        srnd = stat.tile([128, 1], F32, tag="s7", name="s7")
        nc.vector.tensor_scalar(srnd, ra, r, 127.0, OP.mult, OP.mult)
        # round(x * 127/absmax): HW's fp32->int16 convert rounds to nearest
        i16 = work.tile([128, F], I16, tag=f"i16_{F}", name=f"i16_{F}")
        xq = work.tile([128, F], BF16, tag=f"xqtm_{F}", name=f"xqtm_{F}")
        if SIM_COMPAT:
            nc.vector.tensor_scalar(i16, X, srnd, 256.5, OP.mult, OP.add)
            nc.gpsimd.tensor_scalar(xq, i16, -256.0, None, OP.add)
        else:
            nc.vector.tensor_scalar_mul(i16, X, srnd)
            nc.gpsimd.tensor_copy(xq, i16)
        for k in range(FK):
            nc.sync.dma_start(xqT[k][:, j * 128:(j + 1) * 128],
                              xq[:, k * 128:(k + 1) * 128], transpose=True)
    a_bcast = None
    if make_row:
        amat = stat.tile([128, nj], F32, tag="amat", name="amat")
        for j in range(nj):
            nc.gpsimd.tensor_copy(amat[:, j:j + 1], alphas[j])
        pst = g.psum.tile([nj, 128], F32, tag="ps", name="pst")
        nc.tensor.transpose(pst, amat, g.ident)
        at = stat.tile([nj, 128], F32, tag="at", name="at")
        nc.scalar.copy(at, pst)
        arow = stat.tile([1, Ttot], F32, tag="arow", name="arow")
        for j in range(nj):
            nc.sync.dma_start(arow[0:1, j * 128:(j + 1) * 128],
                              at[j:j + 1, :])
        a_bcast = xq_pool.tile([128, Ttot], F32, tag=f"abc_{uid}",
                               name=f"abc_{uid}")
        nc.gpsimd.partition_broadcast(a_bcast, arow[0:1, :])
    return xqT, alphas, a_bcast


def _layernorm(g, a_tiles, g_bc, b_bc, out_tiles):
    nc, stat = g.nc, g.stat
    for j, A in enumerate(a_tiles):
        stats = stat.tile([128, 3, 6], F32, tag="bnst", name="bnst")
        Ag = A.rearrange("p (n s) -> p n s", s=256)
        for gi in range(3):
            nc.vector.bn_stats(stats[:, gi, :], Ag[:, gi, :])
        mv = stat.tile([128, 2], F32, tag="mv", name="mv")
        nc.vector.bn_aggr(mv, stats)
        rs = stat.tile([128, 1], F32, tag="s1", name="s1")
        t0 = stat.tile([128, 1], F32, tag="s2", name="s2")
        _rsqrt(g, rs, mv[:, 1:2], g.eps5, t0)
        X = out_tiles[j]
        nc.vector.tensor_scalar(X, A, mv[:, 0:1], rs, OP.subtract, OP.mult)
        nc.vector.tensor_mul(X, X, g_bc)
        nc.vector.tensor_add(X, X, b_bc)


def _attention(g, s_tiles, kh_tiles, qh_tiles, v_aug, a_out, psum_s, psum_o,
               work):
    """GQA attention. qh_tiles: HQ x [64, T] bf16; kh_tiles: HK x [64, S]
    bf16 (feature-major, base partition 0); v token-major bf16 with an
    appended ones column (softmax denominator via the PV matmul).
    a_out: NT x [128, D] fp32 token-major."""
    nc, stat = g.nc, g.stat
    for h in range(HQ):
        kh = h // 2
        ps_o = psum_o.tile([65, 512], F32, tag="pso", name="pso")
        for s in range(s_tiles):
            ps_s = psum_s.tile([128, 512], F32, tag="pss", name="pss")
            nc.tensor.matmul(ps_s, kh_tiles[kh][0:64, s * 128:(s + 1) * 128],
                             qh_tiles[h][0:64, :], start=True, stop=True)
            pT = work.tile([128, 512], BF16, tag="pT", name="pT")
            nc.scalar.activation(pT, ps_s, ACT.Exp)
            nc.tensor.matmul(ps_o, v_aug[s][:, kh, :], pT,
                             start=(s == 0), stop=(s == s_tiles - 1))
        o_sb = work.tile([65, 512], F32, tag="osb", name="osb")
        nc.scalar.copy(o_sb, ps_o)
        for j in range(NT):
            ps_t = g.psum.tile([128, 65], F32, tag="ps", name="ps_t")
            nc.tensor.transpose(ps_t, o_sb[:, j * 128:(j + 1) * 128],
                                g.ident[0:65, 0:65])
            rec = stat.tile([128, 1], F32, tag="rec", name="rec")
            nc.vector.reciprocal(rec, ps_t[:, 64:65])
            nc.vector.tensor_scalar_mul(a_out[j][:, h * 64:(h + 1) * 64],
                                        ps_t[:, 0:64], rec)


def build_program(groups=None):
    """Build and finalize the SPMD program (identical on all cores)."""
    if groups is None:
        groups = GROUPS
    gsz = len(groups[0])
    n_s = gsz * NT          # gathered key tiles for self-attention
    nc = bacc.Bacc()

    x_in = nc.declare_dram_parameter("x_sh", [T, D], F32, isOutput=False)
    y_in = nc.declare_dram_parameter("y_b", [S, D], F32, isOutput=False)
    wt_in = {}
    for name, (O, I) in WSPECS.items():
        wt_in[name] = nc.declare_dram_parameter(f"{name}_t", [I, O], F32,
                                                isOutput=False)
    ln_in = {}
    for name in ('sa_g', 'sa_b', 'ca_g', 'ca_b'):
        ln_in[name] = nc.declare_dram_parameter(name, [D], F32,
                                                isOutput=False)
    out_sh = nc.declare_dram_parameter("out_sh", [T, D], F32, isOutput=True)

    g = Ctx()
    g.nc = nc

    with tile.TileContext(nc) as tc, ExitStack() as ctx:
        g.tc = tc
        g.const = ctx.enter_context(tc.tile_pool(name="const", bufs=1))
        const = g.const
        g.stat = ctx.enter_context(tc.tile_pool(name="stat", bufs=4))
        g.psum = ctx.enter_context(tc.tile_pool(name="psg", bufs=4,
                                                space="PSUM"))
        dram = ctx.enter_context(tc.tile_pool(name="dram", bufs=1,
                                              space="DRAM"))

        cc_k_in = dram.tile([DKV // 128, 128, T], BF16, name="cc_k_in")
        cc_k_out = dram.tile([gsz, DKV // 128, 128, T], BF16,
                             name="cc_k_out")
        cc_v_in = dram.tile([NT, 128, DKV], BF16, name="cc_v_in")
        cc_v_out = dram.tile([gsz, NT, 128, DKV], BF16, name="cc_v_out")

        g.eps6 = const.tile([128, 1], F32, name="eps6")
        nc.vector.memset(g.eps6, 1e-6)
        g.eps5 = const.tile([128, 1], F32, name="eps5")
        nc.vector.memset(g.eps5, 1e-5)
        g.ones_col = const.tile([128, 1], F32, name="ones_col")
        nc.vector.memset(g.ones_col, 1.0)
        g.ident = const.tile([128, 128], F32, name="ident")
        make_identity(nc, g.ident)

        ln_bc = {}
        for name in ('sa_g', 'sa_b', 'ca_g', 'ca_b'):
            row = const.tile([1, D], F32, tag=f"lnr_{name}",
                             name=f"lnr_{name}")
            nc.sync.dma_start(row[0:1, :],
                              ln_in[name][:].rearrange("(o d) -> o d", o=1))
            bc = const.tile([128, D], F32, tag=f"lnb_{name}",
                            name=f"lnb_{name}")
            nc.gpsimd.partition_broadcast(bc, row[0:1, :])
            ln_bc[name] = bc

        def proj_heads(wsb, xqT, mscale, a_bcast, out_pool, O, Ttot, tag):
            """per-head feature-major projection: O//64 tiles [64, Ttot] bf16
            at base partition 0 (DVE shifts the upper-half partitions)."""
            outs = []
            for mt in range(O // 128):
                ps = g.psum.tile([128, Ttot], F32, tag="ps", name="ps_ph")
                for k in range(len(xqT)):
                    nc.tensor.matmul(ps, wsb[k][:, mt * 128:(mt + 1) * 128],
                                     xqT[k], start=(k == 0),
                                     stop=(k == len(xqT) - 1))
                for half in range(2):
                    o = out_pool.tile([64, Ttot], BF16,
                                      tag=f"{tag}{2 * mt + half}",
                                      name=f"{tag}{2 * mt + half}")
                    sl = slice(half * 64, half * 64 + 64)
                    nc.vector.scalar_tensor_tensor(
                        o[0:64, :], ps[sl, :], mscale[sl, :],
                        a_bcast[sl, :], OP.mult, OP.mult)
                    outs.append(o)
            return outs

        def proj_feat(wsb, xqT, mscale, a_bcast, out_pool, O, Ttot, tag):
            """feature-major projection: O//128 tiles of [128, Ttot] bf16"""
            outs = []
            for mt in range(O // 128):
                ps = g.psum.tile([128, Ttot], F32, tag="ps", name="ps_pf")
                for k in range(len(xqT)):
                    nc.tensor.matmul(ps, wsb[k][:, mt * 128:(mt + 1) * 128],
                                     xqT[k], start=(k == 0),
                                     stop=(k == len(xqT) - 1))
                o = out_pool.tile([128, Ttot], BF16, tag=f"{tag}{mt}",
                                  name=f"{tag}{mt}")
                nc.vector.scalar_tensor_tensor(o, ps, mscale, a_bcast,
                                               OP.mult, OP.mult)
                outs.append(o)
            return outs

        def proj_tok_resid(xqT, wsb, al_list, mb, resid_tiles, out_tiles):
            """token-major projection + dequant + residual-add."""
            for j in range(NT):
                ao = g.stat.tile([128, 1], F32, tag="s1", name="ao")
                nc.vector.tensor_mul(ao, al_list[j], mb)
                for c in range(2):
                    ps = g.psum.tile([128, 384], F32, tag="ps", name="ps_pt")
                    for k in range(KT):
                        nc.tensor.matmul(
                            ps, xqT[k][:, j * 128:(j + 1) * 128],
                            wsb[k][:, c * 384:(c + 1) * 384],
                            start=(k == 0), stop=(k == KT - 1))
                    nc.vector.scalar_tensor_tensor(
                        out_tiles[j][:, c * 384:(c + 1) * 384], ps, ao,
                        resid_tiles[j][:, c * 384:(c + 1) * 384],
                        OP.mult, OP.add)

        resid2 = ctx.enter_context(tc.tile_pool(name="resid2", bufs=1))
        x3 = [resid2.tile([128, D], F32, tag=f"x3_{j}", name=f"x3_{j}")
              for j in range(NT)]

        # ======== Attention phases (SA then CA); weights SBUF-resident ====
        with tc.tile_pool(name="resid1", bufs=1) as resid1, \
             tc.tile_pool(name="wwA", bufs=2) as wwA:
            x2 = [resid1.tile([128, D], F32, tag=f"x2_{j}", name=f"x2_{j}")
                  for j in range(NT)]

            # ---------------- Phase SA ----------------
            with tc.tile_pool(name="attw", bufs=1) as attw:
                wq_sb, m_wq = _prep_weight(g, 'sa_wq', attw, wt_in['sa_wq'], wwA)
                wk_sb, m_wk = _prep_weight(g, 'sa_wk', attw, wt_in['sa_wk'], wwA)
                wv_sb, m_wv = _prep_weight(g, 'sa_wv', attw, wt_in['sa_wv'], wwA)
                wo_sb, m_wo = _prep_weight(g, 'sa_wo', attw, wt_in['sa_wo'], wwA)

                with tc.tile_pool(name="resid0", bufs=1) as resid0, \
                     tc.tile_pool(name="sa_att", bufs=1) as sa_att, \
                     tc.tile_pool(name="sa_work", bufs=3) as work:
                    x_tiles = [resid0.tile([128, D], F32, tag=f"x_{j}",
                                           name=f"x_{j}") for j in range(NT)]
                    for j in range(NT):
                        nc.sync.dma_start(x_tiles[j],
                                          x_in[j * 128:(j + 1) * 128, :])

                    with tc.tile_pool(name="sa_xq1", bufs=1) as sa_xq1:
                        xqT, al_x, abc_x = _act_quant(g, x_tiles, D, sa_xq1,
                                                      work, True, "x1")
                        mq = g.const.tile([128, 1], F32, name="mq_sa")
                        nc.vector.tensor_scalar_mul(mq, m_wq,
                                                    1.0 / float(np.sqrt(HEAD)))
                        qh = proj_heads(wq_sb, xqT, mq, abc_x, sa_att, D, T,
                                        "qh")
                        k_f = proj_feat(wk_sb, xqT, m_wk, abc_x, sa_xq1, DKV,
                                        T, "kf")
                        for t in range(DKV // 128):
                            nc.sync.dma_start(cc_k_in[t, :, :], k_f[t])
                        for j in range(NT):
                            ps = g.psum.tile([128, DKV], F32, tag="ps",
                                             name="ps_v")
                            for k in range(KT):
                                nc.tensor.matmul(
                                    ps, xqT[k][:, j * 128:(j + 1) * 128],
                                    wv_sb[k], start=(k == 0),
                                    stop=(k == KT - 1))
                            av = g.stat.tile([128, 1], F32, tag="s1",
                                             name="av")
                            nc.vector.tensor_mul(av, al_x[j], m_wv)
                            vtok = work.tile([128, DKV], BF16, tag="vtok",
                                             name="vtok")
                            nc.vector.tensor_scalar_mul(vtok, ps, av)
                            nc.sync.dma_start(cc_v_in[j, :, :], vtok)

                        nc.gpsimd.collective_compute(
                            "AllGather", OP.bypass, replica_groups=groups,
                            ins=[cc_k_in[:, :, :].opt()],
                            outs=[cc_k_out[:, :, :, :].opt()])
                        nc.gpsimd.collective_compute(
                            "AllGather", OP.bypass, replica_groups=groups,
                            ins=[cc_v_in[:, :, :].opt()],
                            outs=[cc_v_out[:, :, :, :].opt()])

                    with tc.tile_pool(name="sa_kv", bufs=1) as sa_kv, \
                         tc.tile_pool(name="sa_a", bufs=1) as sa_a, \
                         tc.tile_pool(name="ps_s", bufs=2,
                                      space="PSUM") as psum_s, \
                         tc.tile_pool(name="ps_o", bufs=2,
                                      space="PSUM") as psum_o:
                        kh_tiles = []
                        for kh in range(HK):
                            kt = sa_kv.tile([64, n_s * 128], BF16,
                                            tag=f"kT{kh}", name=f"kT{kh}")
                            srcp = cc_k_out[:, kh // 2,
                                            (kh % 2) * 64:(kh % 2) * 64 + 64,
                                            :]
                            nc.sync.dma_start(
                                kt[0:64, :].rearrange("p (r t) -> p r t",
                                                      r=gsz),
                                srcp.transpose([1, 0, 2]))
                            kh_tiles.append(kt)
                        v_aug = []
                        for s in range(n_s):
                            r, j = s // NT, s % NT
                            va = sa_kv.tile([128, HK, HEAD + 1], BF16,
                                            tag=f"va{s}", name=f"va{s}")
                            nc.sync.dma_start(
                                va[:, :, 0:HEAD],
                                cc_v_out[r, j, :, :].rearrange(
                                    "p (h e) -> p h e", e=HEAD))
                            nc.vector.memset(va[:, :, HEAD:HEAD + 1], 1.0)
                            v_aug.append(va)

                        a_tok = [sa_a.tile([128, D], F32, tag=f"a{j}",
                                           name=f"a{j}") for j in range(NT)]
                        _attention(g, n_s, kh_tiles, qh, v_aug, a_tok,
                                   psum_s, psum_o, work)

                        ln_t = [sa_a.tile([128, D], F32, tag=f"l{j}",
                                          name=f"l{j}") for j in range(NT)]
                        _layernorm(g, a_tok, ln_bc['sa_g'], ln_bc['sa_b'],
                                   ln_t)
                        aqT, al_a, _ = _act_quant(g, ln_t, D, sa_a, work,
                                                  False, "a1")
                        proj_tok_resid(aqT, wo_sb, al_a, m_wo, x_tiles, x2)

            # ---------------- Phase CA ----------------
            with tc.tile_pool(name="caw", bufs=1) as caw, \
                 tc.tile_pool(name="ca_xq", bufs=1) as ca_xq, \
                 tc.tile_pool(name="ca_misc", bufs=1) as ca_misc, \
                 tc.tile_pool(name="ca_work", bufs=3) as work:
                wqc_sb, m_wqc = _prep_weight(g, 'ca_wq', caw, wt_in['ca_wq'],
                                             wwA)
                wkc_sb, m_wkc = _prep_weight(g, 'ca_wk', caw, wt_in['ca_wk'],
                                             wwA)
                wvc_sb, m_wvc = _prep_weight(g, 'ca_wv', caw, wt_in['ca_wv'],
                                             wwA)
                woc_sb, m_woc = _prep_weight(g, 'ca_wo', caw, wt_in['ca_wo'],
                                             wwA)
                wc_sb, m_wc = _prep_weight(g, 'w_cond', caw, wt_in['w_cond'],
                                           wwA)
                y_tiles = [ca_misc.tile([128, D], F32, tag=f"y_{j}",
                                        name=f"y_{j}") for j in range(ST)]
                for j in range(ST):
                    nc.sync.dma_start(y_tiles[j],
                                      y_in[j * 128:(j + 1) * 128, :])
                yqT, al_y, _ = _act_quant(g, y_tiles, D, ca_xq, work, False,
                                          "y")
                yc = [ca_misc.tile([128, D], F32, tag=f"yc_{j}",
                                   name=f"yc_{j}") for j in range(ST)]
                for j in range(ST):
                    am = g.stat.tile([128, 1], F32, tag="s1", name="am")
                    nc.vector.tensor_mul(am, al_y[j], m_wc)
                    for c in range(2):
                        ps = g.psum.tile([128, 384], F32, tag="ps",
                                         name="ps_yc")
                        for k in range(KT):
                            nc.tensor.matmul(
                                ps, yqT[k][:, j * 128:(j + 1) * 128],
                                wc_sb[k][:, c * 384:(c + 1) * 384],
                                start=(k == 0), stop=(k == KT - 1))
                        nc.vector.tensor_scalar_mul(
                            yc[j][:, c * 384:(c + 1) * 384], ps, am)

                ycqT, al_yc, abc_yc = _act_quant(g, yc, D, ca_xq, work,
                                                 True, "yc")
                x2qT, al_x2, abc_x2 = _act_quant(g, x2, D, ca_xq, work,
                                                 True, "x2")

                with tc.tile_pool(name="ca_kv", bufs=1) as ca_kv, \
                     tc.tile_pool(name="ca_a", bufs=1) as ca_a, \
                     tc.tile_pool(name="ps_s2", bufs=2,
                                  space="PSUM") as psum_s, \
                     tc.tile_pool(name="ps_o2", bufs=2,
                                  space="PSUM") as psum_o:
                    mqc = g.const.tile([128, 1], F32, name="mq_ca")
                    nc.vector.tensor_scalar_mul(mqc, m_wqc,
                                                1.0 / float(np.sqrt(HEAD)))
                    q2h = proj_heads(wqc_sb, x2qT, mqc, abc_x2, ca_kv, D, T,
                                     "q2h")
                    kch = proj_heads(wkc_sb, ycqT, m_wkc, abc_yc, ca_kv,
                                     DKV, S, "kch")
                    v_ca = []
                    for j in range(ST):
                        ps = g.psum.tile([128, DKV], F32, tag="ps",
                                         name="ps_vc")
                        for k in range(KT):
                            nc.tensor.matmul(
                                ps, ycqT[k][:, j * 128:(j + 1) * 128],
                                wvc_sb[k], start=(k == 0),
                                stop=(k == KT - 1))
                        av = g.stat.tile([128, 1], F32, tag="s1", name="avc")
                        nc.vector.tensor_mul(av, al_yc[j], m_wvc)
                        va = ca_kv.tile([128, HK, HEAD + 1], BF16,
                                        tag=f"vc{j}", name=f"vc{j}")
                        nc.vector.tensor_scalar_mul(
                            va[:, :, 0:HEAD],
                            ps.rearrange("p (h e) -> p h e", e=HEAD), av)
                        nc.vector.memset(va[:, :, HEAD:HEAD + 1], 1.0)
                        v_ca.append(va)

                    a2_tok = [ca_a.tile([128, D], F32, tag=f"a{j}",
                                        name=f"a{j}") for j in range(NT)]
                    _attention(g, ST, kch, q2h, v_ca, a2_tok, psum_s,
                               psum_o, work)

                    ln2 = [ca_a.tile([128, D], F32, tag=f"l{j}",
                                     name=f"l{j}") for j in range(NT)]
                    _layernorm(g, a2_tok, ln_bc['ca_g'], ln_bc['ca_b'], ln2)
                    a2qT, al_a2, _ = _act_quant(g, ln2, D, ca_a, work,
                                                False, "a2")
                    proj_tok_resid(a2qT, woc_sb, al_a2, m_woc, x2, x3)

        # ================= Phase FFN =================
        with tc.tile_pool(name="ffn_xq", bufs=1) as ffn_xq, \
             tc.tile_pool(name="ffn_work", bufs=2) as work, \
             tc.tile_pool(name="wwF", bufs=2) as wwF:
            x3qT, al_3, _ = _act_quant(g, x3, D, ffn_xq, work, False, "x3")
            with tc.tile_pool(name="ffn_h", bufs=1) as ffn_h:
                h_bf = [ffn_h.tile([128, H4], BF16, tag=f"h{j}",
                                   name=f"h{j}") for j in range(NT)]
                with tc.tile_pool(name="w1p", bufs=1) as w1p:
                    w1_sb, m_w1 = _prep_weight(g, 'w1', w1p, wt_in['w1'],
                                               wwF, two_pass=True)
                    for j in range(NT):
                        a3 = g.stat.tile([128, 1], F32, tag=f"a3_{j}",
                                         name=f"a3_{j}")
                        nc.vector.tensor_mul(a3, al_3[j], m_w1)
                        for c in range(6):
                            ps = g.psum.tile([128, 512], F32, tag="ps",
                                             name="ps_h")
                            for k in range(KT):
                                nc.tensor.matmul(
                                    ps, x3qT[k][:, j * 128:(j + 1) * 128],
                                    w1_sb[k][:, c * 512:(c + 1) * 512],
                                    start=(k == 0), stop=(k == KT - 1))
                            # fused dequant + exact (erf) GELU
                            nc.scalar.activation(
                                h_bf[j][:, c * 512:(c + 1) * 512], ps,
                                ACT.Gelu, bias=0.0, scale=a3)

                hqT, al_h, _ = _act_quant(g, h_bf, H4, ffn_xq, work, False,
                                          "h")
            with tc.tile_pool(name="w2p", bufs=1) as w2p, \
                 tc.tile_pool(name="outp", bufs=2) as outp:
                w2_sb, m_w2 = _prep_weight(g, 'w2', w2p, wt_in['w2'], wwF,
                                           two_pass=True)
                for j in range(NT):
                    ah = g.stat.tile([128, 1], F32, tag="s1", name="ah")
                    nc.vector.tensor_mul(ah, al_h[j], m_w2)
                    xo = outp.tile([128, D], F32, tag="xo", name="xo")
                    for c in range(2):
                        ps = g.psum.tile([128, 384], F32, tag="ps",
                                         name="ps_w2")
                        for k in range(KTH):
                            nc.tensor.matmul(
                                ps, hqT[k][:, j * 128:(j + 1) * 128],
                                w2_sb[k][:, c * 384:(c + 1) * 384],
                                start=(k == 0), stop=(k == KTH - 1))
                        nc.vector.scalar_tensor_tensor(
                            xo[:, c * 384:(c + 1) * 384], ps, ah,
                            x3[j][:, c * 384:(c + 1) * 384], OP.mult,
                            OP.add)
                    nc.sync.dma_start(out_sh[j * 128:(j + 1) * 128, :], xo)

    nc.finalize()
    return nc


def _get_program(key="full"):
    if key not in _PROGRAM_CACHE:
        _PROGRAM_CACHE[key] = build_program(
            GROUPS if key == "full" else [[0]])
    return _PROGRAM_CACHE[key]


LAST_RESULT = None


def kernel(**inputs):
    """Full-input entry: shard across 8 cores, run, gather."""
    global LAST_RESULT
    nc = _get_program()
    x = np.ascontiguousarray(np.asarray(inputs['x'], dtype=np.float32))
    y = np.ascontiguousarray(np.asarray(inputs['y'], dtype=np.float32))
    common = {}
    for name in WSPECS:
        common[f"{name}_t"] = np.ascontiguousarray(
            np.asarray(inputs[name], np.float32).T)
    for name in ('sa_g', 'sa_b', 'ca_g', 'ca_b'):
        common[name] = np.ascontiguousarray(
            np.asarray(inputs[name], np.float32))
    in_maps = []
    for c in range(NCORES):
        b, seg = c // GSZ, c % GSZ
        m = dict(common)
        m['x_sh'] = np.ascontiguousarray(x[b, seg * T:(seg + 1) * T, :])
        m['y_b'] = np.ascontiguousarray(y[b])
        in_maps.append(m)
    res = run_bass_kernel_spmd(nc, in_maps, core_ids=list(range(NCORES)))
    LAST_RESULT = res
    out = np.empty((B, N, D), np.float32)
    for c in range(NCORES):
        b, seg = c // GSZ, c % GSZ
        out[b, seg * T:(seg + 1) * T, :] = res.results[c]['out_sh']
    return out

